# revision 33
# baseline (speedup 1.0000x reference)
"""Trainium2 Bass kernel for a cross-attention block.

reference semantics (jax):
    q = x @ Wq + bq                      # (b, hw, c)
    k = p @ Wk + bk                      # (b, 77, c)
    v = p @ Wv + bv                      # (b, 77, c)
    scores = einsum("bqhd,bkhd->bhqk", q, k) / sqrt(hd)
    attn = softmax(scores, -1)
    out = einsum("bhqk,bkhd->bqhd", attn, v) @ Ww + bw

Sharding: data-parallel over batch (16 batches / 8 cores = 2 per core), no
collectives.  Per core everything runs "features-on-partitions" so every
matmul contraction lands on SBUF partitions:

    X^T (PE transpose) -> Q^T = Wq^T @ X^T (bf16, fp32 PSUM)
    scores^T[77, 512] = kT_h.T @ qT_h  per head (pairs row-packed in the PE)
    exp on ScalarE straight out of PSUM (scale=1/8 folded in)
    den[h] for all 16 heads accumulated into ONE [16, 512] PSUM tile via
      one-hot selector matmuls; a single DVE reciprocal per chunk
    1/den broadcast to [128, 512] via a PE selector matmul (kron(I16, 1_64))
    aT = attnV result * bc  (DVE, normalizes while moving PSUM->SBUF)
    out[hw, c] = aT.T @ Ww + bw  (natural layout -> contiguous store)

Performance structure (1.64 ms baseline -> 0.91 ms):
 - F=512 chunks: every matmul group fills one PSUM bank (N=512), halving
   instruction count; PSUM matmul dsts must be full-bank-width (packed).
 - all PE inputs bf16 (weights/xT/qT/kT/v/ex/aT/selectors), fp32 PSUM.
 - 3-stage pipeline across chunks with fine-grained emission interleaving:
   next chunk's transposes weave into the exp-limited scores front, the
   reciprocal latency is hidden under Q-proj + out-proj groups, and each
   attnV PSUM bank is normalized immediately so 2 bufs per pool suffice.
 - denominator bank freed early via a ScalarE copy; the slow DVE
   reciprocal reads the SBUF copy instead.
 - first x chunk's DMA + wk/wv loads lead the queue (wq/ww loaded after
   phase A), and a junk-matmul warm-up burst keeps the PE HAM-warm from
   the start.
"""

import numpy as np
from contextlib import ExitStack

import concourse.bass as bass
import concourse.tile as tile
from concourse import bacc, mybir
from concourse.bass_utils import run_bass_kernel_spmd
from concourse.masks import make_identity

N_CORES = 8
B_FULL, HW, C = 16, 4096, 1024
NH, D, CTX, NE = 16, 64, 77, 512
B = B_FULL // N_CORES          # batches per core
P = 128
KC = C // P                    # 8 c-chunks of 128
KN = NE // P                   # 4 n_embd chunks of 128
F = 512                        # hw elements per chunk (= 1 PSUM bank fp32)
FSUB = F // P                  # 128-row subchunks per chunk
NPAIR = NH // 2                # 8 head pairs (pair p == aT tile p)

F32 = mybir.dt.float32
BF16 = mybir.dt.bfloat16
EXP = mybir.ActivationFunctionType.Exp
IDENT = mybir.ActivationFunctionType.Identity
COPY = mybir.ActivationFunctionType.Copy


def _bcast_dram(ap, parts, free):
    """DRAM 1-D tensor broadcast across `parts` partitions (step-0 AP)."""
    return bass.AP(tensor=ap.tensor, offset=ap.offset, ap=[[0, parts], [1, free]])


def _body(ctx: ExitStack, tc: tile.TileContext, io: dict, hw: int = HW):
    nc = tc.nc
    nchunk = hw // F

    x_ap, p_ap, out_ap = io["x"], io["p"], io["out"]
    wq_ap, bq_ap = io["Wq"], io["bq"]
    wk_ap, bk_ap = io["Wk"], io["bk"]
    wv_ap, bv_ap = io["Wv"], io["bv"]
    ww_ap, bw_ap = io["Ww"], io["bw"]

    # ---------------- pools ----------------
    consts = ctx.enter_context(tc.tile_pool(name="consts", bufs=1))
    wpool = ctx.enter_context(tc.tile_pool(name="wpool", bufs=1))
    kvout = ctx.enter_context(tc.tile_pool(name="kvout", bufs=1))
    # PSUM: 4 pools x 2 bufs x [128,512] = all 8 banks
    ps_tp = ctx.enter_context(tc.tile_pool(name="ps_tp", bufs=2, space="PSUM"))
    ps_acc = ctx.enter_context(tc.tile_pool(name="ps_acc", bufs=2, space="PSUM"))
    ps_at = ctx.enter_context(tc.tile_pool(name="ps_at", bufs=2, space="PSUM"))
    ps_pr = ctx.enter_context(tc.tile_pool(name="ps_pr", bufs=2, space="PSUM"))

    # ---------------- constants ----------------
    ident = consts.tile([P, P], F32, name="ident")
    make_identity(nc, ident[:])

    bq_sb = consts.tile([P, KC], F32, name="bq_sb")
    nc.sync.dma_start(out=bq_sb[:], in_=bq_ap.rearrange("(a b) -> b a", b=P))
    bk_sb = consts.tile([P, KC], F32, name="bk_sb")
    nc.sync.dma_start(out=bk_sb[:], in_=bk_ap.rearrange("(a b) -> b a", b=P))
    bv_bc = consts.tile([CTX, C], F32, name="bv_bc")
    nc.sync.dma_start(out=bv_bc[:], in_=_bcast_dram(bv_ap, CTX, C))
    bw_bc = consts.tile([P, C], F32, name="bw_bc")
    nc.sync.dma_start(out=bw_bc[:], in_=_bcast_dram(bw_ap, P, C))

    # one-hot selector for denominator accumulation: sel[:, 16h + h] = 1
    # so sel[:, 16h:16h+16] is ones in column h -> den matmul h writes row h
    scratch = consts.tile([CTX, NH * NH], F32, name="scratch")
    nc.vector.memset(scratch[:], 0.0)
    nc.vector.memset(scratch[:, 0 : NH * NH : NH + 1], 1.0)
    sel = consts.tile([CTX, NH * NH], BF16, name="sel")
    nc.vector.tensor_copy(out=sel[:], in_=scratch[:])

    # selector for inv broadcast: bc_p[r, :] = inv16[2p + (r>=64), :]
    # selb[h, p*128+r] = 1 iff h == 2p + (r >= 64), i.e. cols [64h, 64h+64)
    # == kron(I_16, ones(64)): broadcast-copy of the identity's 16x16 corner
    selb = consts.tile([NH, NPAIR * P], BF16, name="selb")
    nc.vector.tensor_copy(
        out=selb[:].rearrange("h (c j) -> h c j", j=D),
        in_=ident[0:NH, 0:NH].to_broadcast([NH, NH, D]),
    )

    # x pool + first chunk's DMA before the 3MB of weight loads, so the
    # pipeline's first transposes aren't queued behind them
    xpool = ctx.enter_context(tc.tile_pool(name="xpool", bufs=1))
    nC = B * (hw // F)
    st = {}  # per-chunk pipeline state

    def a_dma(ci):
        b, j = divmod(ci, hw // F)
        r0 = j * F
        xn = []
        for r in range(FSUB):
            t = xpool.tile([P, C], F32, name="xn", tag="xn", bufs=8)
            nc.sync.dma_start(
                out=t[:], in_=x_ap[b, r0 + r * P : r0 + (r + 1) * P, :]
            )
            xn.append(t)
        st[ci] = {"xn": xn, "xT": [], "qT": [], "exs": [], "aT": []}

    a_dma(0)

    # warm-up burst: junk matmuls (ident @ ident) keep the PE busy past the
    # HAM activity window (~3.4us) so the real stream starts at full clock;
    # they run during the startup DMA wait and delay nothing
    warm_t = ps_acc.tile([P, P], F32, name="warm", tag="acc")
    for w in range(24):
        nc.tensor.matmul(
            warm_t[:],
            ident[:],
            ident[:],
            start=(w == 0),
            stop=(w == 23),
        )

    # K^T tiles [128, 77] per (batch, c-chunk); V natural [77, 1024]
    kT = [
        [kvout.tile([P, CTX], BF16, name=f"kT{b}_{m}", tag=f"kT{b}_{m}") for m in range(KC)]
        for b in range(B)
    ]
    v2 = [
        kvout.tile([CTX, C], BF16, name=f"v2_{b}", tag=f"v2_{b}") for b in range(B)
    ]

    # ---------------- phase A: K/V projections (tiny) ----------------
    with ExitStack() as kvctx:
        wkv = kvctx.enter_context(tc.tile_pool(name="wkv", bufs=1))
        ppool = kvctx.enter_context(tc.tile_pool(name="ppool", bufs=2))
        wk = []
        wv = []
        for k in range(KN):
            s = kvctx.enter_context(tc.tile_pool(name=f"wst{k}", bufs=1))
            st1 = s.tile([P, C], F32, name="wks")
            nc.sync.dma_start(out=st1[:], in_=wk_ap[k * P : (k + 1) * P, :])
            t = wkv.tile([P, C], BF16, name=f"wk{k}", tag=f"wk{k}")
            nc.vector.tensor_copy(out=t[:], in_=st1[:])
            wk.append(t)
            st2 = s.tile([P, C], F32, name="wvs")
            nc.sync.dma_start(out=st2[:], in_=wv_ap[k * P : (k + 1) * P, :])
            t = wkv.tile([P, C], BF16, name=f"wv{k}", tag=f"wv{k}")
            nc.vector.tensor_copy(out=t[:], in_=st2[:])
            wv.append(t)

        for b in range(B):
            pnat = ppool.tile([CTX, NE], F32, name="pnat", tag="pnat", bufs=2)
            nc.sync.dma_start(out=pnat[:], in_=p_ap[b])
            # PE-transpose into pT[k] [128, 77] (transpose-mode matmuls may
            # write column slices of a bank)
            pT = []
            for g in range(2):
                tp_t = ps_tp.tile([P, F], F32, name="ps_pT", tag="tp")
                for kq in range(2):
                    k = g * 2 + kq
                    nc.tensor.transpose(
                        tp_t[:, kq * P : kq * P + CTX],
                        pnat[:, k * P : (k + 1) * P],
                        ident[:CTX, :CTX],
                    )
                for kq in range(2):
                    k = g * 2 + kq
                    t = ppool.tile([P, CTX], BF16, name=f"pT{k}", tag=f"pT{k}", bufs=2)
                    nc.vector.tensor_copy(
                        out=t[:], in_=tp_t[:, kq * P : kq * P + CTX]
                    )
                    pT.append(t)

            # K^T[mc] = sum_k Wk[k,mc-slice].T @ pT[k]  (+ bk)
            for mc in range(KC):
                at_t = ps_at.tile([P, CTX], F32, name="ps_kT", tag="at")
                for k in range(KN):
                    nc.tensor.matmul(
                        at_t[:],
                        wk[k][:, mc * P : (mc + 1) * P],
                        pT[k][:],
                        start=(k == 0),
                        stop=(k == KN - 1),
                    )
                nc.vector.tensor_add(
                    kT[b][mc][:],
                    at_t[:],
                    bk_sb[:, mc : mc + 1].to_broadcast([P, CTX]),
                )

            # V natural [77, c]: lhsT = pT[k] (K=128, M=77), rhs = Wv slice
            for nb in range(C // 512):
                pr_t = ps_pr.tile([CTX, 512], F32, name="ps_v", tag="pr")
                for k in range(KN):
                    nc.tensor.matmul(
                        pr_t[:],
                        pT[k][:],
                        wv[k][:, nb * 512 : (nb + 1) * 512],
                        start=(k == 0),
                        stop=(k == KN - 1),
                    )
                nc.vector.tensor_add(
                    v2[b][:, nb * 512 : (nb + 1) * 512],
                    pr_t[:],
                    bv_bc[:, nb * 512 : (nb + 1) * 512],
                )

    # resident weights, cast to bf16: Wq / Ww as 8 [128, 1024] k-slices.
    # Loaded AFTER phase A so the small wk/wv + p + x(0) DMAs lead the queue
    # and the PE can start within ~15us instead of ~58us.
    wq = []
    ww = []
    for k in range(KC):
        s = consts.tile([P, C], F32, name="wqs", tag="wstage", bufs=4)
        nc.sync.dma_start(out=s[:], in_=wq_ap[k * P : (k + 1) * P, :])
        t = wpool.tile([P, C], BF16, name=f"wq{k}", tag=f"wq{k}")
        nc.vector.tensor_copy(out=t[:], in_=s[:])
        wq.append(t)
    for k in range(KC):
        s = consts.tile([P, C], F32, name="wws", tag="wstage", bufs=4)
        nc.sync.dma_start(out=s[:], in_=ww_ap[k * P : (k + 1) * P, :])
        t = wpool.tile([P, C], BF16, name=f"ww{k}", tag=f"ww{k}")
        nc.vector.tensor_copy(out=t[:], in_=s[:])
        ww.append(t)

    # ---------------- phase B: pipelined main loop ----------------
    qpool = ctx.enter_context(tc.tile_pool(name="qpool", bufs=1))
    apool = ctx.enter_context(tc.tile_pool(name="apool", bufs=1))
    epool = ctx.enter_context(tc.tile_pool(name="epool", bufs=18))
    opool = ctx.enter_context(tc.tile_pool(name="opool", bufs=6))
    spool = ctx.enter_context(tc.tile_pool(name="spool", bufs=4))

    # ---- stage pieces (emission-interleaved across 3 pipeline stages) ----

    def a_transpose(ci, kc):
        """4 [128,128] PE transposes -> one bank; DVE copy casts to bf16."""
        xn = st[ci]["xn"]
        tp_t = ps_tp.tile([P, F], F32, name="ps_xT", tag="tp")
        for r in range(FSUB):
            nc.tensor.transpose(
                tp_t[:, r * P : (r + 1) * P],
                xn[r][:, kc * P : (kc + 1) * P],
                ident[:],
            )
        t = xpool.tile([P, F], BF16, name="xT", tag="xT", bufs=10)
        nc.vector.tensor_copy(out=t[:], in_=tp_t[:])
        st[ci]["xT"].append(t)

    def a_qproj(ci, mc):
        xT = st[ci]["xT"]
        acc_t = ps_acc.tile([P, F], F32, name="ps_qT", tag="acc")
        for kc in range(KC):
            nc.tensor.matmul(
                acc_t[:],
                wq[kc][:, mc * P : (mc + 1) * P],
                xT[kc][:],
                start=(kc == 0),
                stop=(kc == KC - 1),
            )
        t = qpool.tile([P, F], BF16, name="qT", tag="qT", bufs=16)
        nc.scalar.activation(t[:], acc_t[:], IDENT, bias=bq_sb[:, mc : mc + 1])
        st[ci]["qT"].append(t)

    def b_score(ci, h):
        b, _ = divmod(ci, nchunk)
        p, hq = divmod(h, 2)
        half = hq * D
        at_t = ps_at.tile([CTX, F], F32, name="ps_s", tag="at")
        # scores^T [77, F]; pair row-packed in the PE (concurrent)
        nc.tensor.matmul(
            at_t[:],
            kT[b][p][half : half + D, :],
            st[ci]["qT"][p][half : half + D, :],
            start=True,
            stop=True,
            tile_position=(half, 0),
        )
        ex = epool.tile([CTX, F], BF16, name="ex", tag="ex", bufs=18)
        nc.scalar.activation(ex[:], at_t[:], EXP, scale=0.125)
        st[ci]["exs"].append(ex)

    def b_den(ci, h):
        # denominator row h accumulates into den_t via one-hot selector.
        # Each den matmul is a closed single-instruction group accumulating
        # via PSUM has_written semantics (an open 16-matmul group would
        # deadlock the scheduler: groups must be contiguous on the PE).
        nc.tensor.matmul(
            st[ci]["den_t"][:],
            sel[:, h * NH : (h + 1) * NH],
            st[ci]["exs"][h][:],
            start=(h == 0),
            stop=True,
            skip_group_check=True,
        )

    def b_recip(ci):
        # free the den PSUM bank quickly with a ScalarE copy, then the slow
        # DVE reciprocal reads SBUF so the bank can be reused meanwhile
        den_sb = spool.tile([NH, F], F32, name="den_sb", tag="den", bufs=2)
        nc.scalar.activation(den_sb[:], st[ci]["den_t"][:], COPY)
        inv16 = spool.tile([NH, F], BF16, name="inv16", tag="inv", bufs=2)
        with nc.allow_low_precision(reason="1/den rounded to bf16 for PE bcast"):
            nc.vector.reciprocal(out=inv16[:], in_=den_sb[:])
        st[ci]["inv16"] = inv16
        del st[ci]["den_t"]

    def b_pair(ci, p):
        b, _ = divmod(ci, nchunk)
        # PE selector-matmul broadcast: bc_p [128, F] rows <- inv16 rows
        bc_ps = ps_at.tile([P, F], F32, name="ps_bc", tag="at")
        nc.tensor.matmul(
            bc_ps[:],
            selb[:, p * P : (p + 1) * P],
            st[ci]["inv16"][:],
            start=True,
            stop=True,
        )
        bc = spool.tile([P, F], F32, name="bc", tag="bc", bufs=4)
        # DVE, not ScalarE: keeps the next chunk's exps at the front of the
        # ScalarE queue (they gate the scores-bank rotation), and the
        # consuming mul is on the DVE anyway
        nc.vector.tensor_copy(out=bc[:], in_=bc_ps[:])
        t = apool.tile([P, F], BF16, name="aT", tag="aT", bufs=16)
        # attnV per head (PSUM matmul dst must start at partition 0),
        # normalize into the pair's aT halves right away so pr recycles
        for hq in range(2):
            h = 2 * p + hq
            half = hq * D
            pr_t = ps_pr.tile([D, F], F32, name="ps_pr", tag="pr")
            nc.tensor.matmul(
                pr_t[:],
                v2[b][:, h * D : (h + 1) * D],
                st[ci]["exs"][h][:],
                start=True,
                stop=True,
            )
            nc.vector.tensor_mul(
                t[half : half + D, :], pr_t[:], bc[half : half + D, :]
            )
        st[ci]["aT"].append(t)

    def c_group(ci, fs):
        b, j = divmod(ci, nchunk)
        r0 = j * F
        aT = st[ci]["aT"]
        osb = opool.tile([P, C], F32, name="osb", tag="osb")
        for nb in range(C // 512):
            f_t = ps_acc.tile([P, 512], F32, name="ps_f", tag="acc")
            for kc in range(KC):
                nc.tensor.matmul(
                    f_t[:],
                    aT[kc][:, fs * P : (fs + 1) * P],
                    ww[kc][:, nb * 512 : (nb + 1) * 512],
                    start=(kc == 0),
                    stop=(kc == KC - 1),
                )
            nc.vector.tensor_add(
                osb[:, nb * 512 : (nb + 1) * 512],
                f_t[:],
                bw_bc[:, nb * 512 : (nb + 1) * 512],
            )
        nc.sync.dma_start(
            out=out_ap[b, r0 + fs * P : r0 + (fs + 1) * P, :], in_=osb[:]
        )

    # ---- slot loop: A = chunk s (load/Q), B = s-1 (attention), C = s-2 ----
    for s in range(nC + 2):
        A, Bc, Cc = s, s - 1, s - 2
        hasA, hasB, hasC = A < nC, 0 <= Bc < nC, 0 <= Cc < nC
        if A + 1 < nC:
            a_dma(A + 1)
        if hasB:
            # front: scores/exp exp-rate-limited; weave in A's transposes and
            # delay each den matmul 2 heads so it never stalls the PE
            st[Bc]["den_t"] = ps_acc.tile([NH, F], F32, name="ps_den", tag="acc")
            for h in range(NH):
                b_score(Bc, h)
                if h >= 2:
                    b_den(Bc, h - 2)
                if h % 2 == 1 and hasA:
                    a_transpose(A, h // 2)
            b_den(Bc, NH - 2)
            b_den(Bc, NH - 1)
            b_recip(Bc)
        elif hasA:
            for kc in range(KC):
                a_transpose(A, kc)
        # mid: Q groups + C out-proj fill the reciprocal latency; B pairs
        # (which need inv16) start 2 iterations later
        for i in range(10):
            if i < 8:
                if hasA:
                    a_qproj(A, i)
                if hasC and i < 4:
                    c_group(Cc, i)
            if 2 <= i and hasB:
                b_pair(Bc, i - 2)
        if hasC:
            del st[Cc]

def build_program(hw: int = HW):
    """Build + compile the per-core Bass program (SPMD, identical per core)."""
    nc = bacc.Bacc(
        "TRN2", target_bir_lowering=False, debug=False, num_devices=N_CORES
    )
    io = {}
    io["x"] = nc.dram_tensor("x", [B, hw, C], F32, kind="ExternalInput").ap()
    io["p"] = nc.dram_tensor("p", [B, CTX, NE], F32, kind="ExternalInput").ap()
    for name, shape in [
        ("Wq", [C, C]),
        ("bq", [C]),
        ("Wk", [NE, C]),
        ("bk", [C]),
        ("Wv", [NE, C]),
        ("bv", [C]),
        ("Ww", [C, C]),
        ("bw", [C]),
    ]:
        io[name] = nc.dram_tensor(name, shape, F32, kind="ExternalInput").ap()
    io["out"] = nc.dram_tensor("out", [B, hw, C], F32, kind="ExternalOutput").ap()

    with tile.TileContext(nc) as tc:
        with ExitStack() as ctx:
            _body(ctx, tc, io, hw=hw)
    nc.compile()
    return nc


_PROGRAM = None


def run_sharded(inputs: dict, trace: bool = False, **trace_kwargs):
    """Shard inputs over the 8 cores, run, gather. Returns (out, results)."""
    global _PROGRAM
    if _PROGRAM is None:
        _PROGRAM = build_program()
    nc = _PROGRAM

    full = {
        k: np.ascontiguousarray(v, dtype=np.float32)
        for k, v in inputs.items()
    }
    in_maps = []
    for i in range(N_CORES):
        m = dict(full)
        m["x"] = full["x"][i * B : (i + 1) * B]
        m["p"] = full["p"][i * B : (i + 1) * B]
        in_maps.append(m)

    res = run_bass_kernel_spmd(
        nc, in_maps, list(range(N_CORES)), trace=trace, **trace_kwargs
    )
    out = np.concatenate([res.results[i]["out"] for i in range(N_CORES)], axis=0)
    return out, res


def kernel(x, p, Wq, bq, Wk, bk, Wv, bv, Ww, bw):
    out, _ = run_sharded(
        dict(x=x, p=p, Wq=Wq, bq=bq, Wk=Wk, bk=bk, Wv=Wv, bv=bv, Ww=Ww, bw=bw)
    )
    return out
